# revision 8
# baseline (speedup 1.0000x reference)
"""MiniRocket-style dilated conv features on Trainium2 (Bass/Tile).

Problem: x[16,12,5000] f32, per-dilation ternary weight banks
weights[10,1000,12,9], biases[10,1000].  For each dilation d in
[1,2,...,512]: y = conv1d(x, W_d, rhs_dilation=d, SAME) -> [B,1000,5000];
features are max over time and PPV (mean of y > bias) -> [16, 20000].

Strategy (8 NeuronCores, data-parallel over batch, 2 batches/core):
  - Build a 108-row shifted-input stack Xs[(j,c), t] = x[c, t+(j-4)d]
    (zero padded) in SBUF via DMA, fp16.
  - Conv as TensorE matmuls: out[k, t] = sum_r W^T[r, k] * Xs[r, t],
    contract dim 108, M=125 kernels/tile, N=512 cols/matmul -> fp32 PSUM.
  - Reductions straight off PSUM:
      * ScalarE evicts most chunks PSUM f32 -> SBUF fp16 (ACTIVATE Copy).
      * VectorE tensor_scalar(+accum) does max-reduce at 4x on the fp16
        copies; the tail chunk is evicted+max-reduced by VectorE itself
        (fused, 1x from PSUM) to balance engine load.
      * PPV via tensor_scalar(is_gt bias, accum add) at 4x on fp16.
  - Tiny final merges (reduce over 3 chunk slots) + DMA out.

Host-side prep is layout only: fp16 casts and the W -> W^T[(j,c),k]
transpose.
"""

import numpy as np

import concourse.bacc as bacc
import concourse.mybir as mybir
import concourse.tile as tile
from concourse.bass_utils import run_bass_kernel_spmd

L = 5000
C = 12
KLEN = 9
DILS = [1, 2, 4, 8, 16, 32, 64, 128, 256, 512]
ND = len(DILS)
KPD = 1000
NKT = 8          # kernel tiles per dilation
MT = 125         # kernels per tile (psum partition dim)
NB = 2           # batches per core
NCORES = 8
CONTRACT = C * KLEN  # 108
MM_N = 512
CHUNKS = [(0, 2048), (2048, 4096), (4096, 5000)]
FP16 = mybir.dt.float16
F32 = mybir.dt.float32
ALU = mybir.AluOpType


def _emit(nc):
    xh = nc.dram_tensor("xh", [NB, C, L], FP16, kind="ExternalInput")
    wt = nc.dram_tensor("wt", [ND, CONTRACT, KPD], FP16, kind="ExternalInput")
    bia = nc.dram_tensor("bia", [ND, KPD], F32, kind="ExternalInput")
    zer = nc.dram_tensor("zer", [C, 2048], FP16, kind="ExternalInput")
    out = nc.dram_tensor("out", [NB, 2 * ND * KPD], F32, kind="ExternalOutput")

    with tile.TileContext(nc) as tc:
        with (
            tc.tile_pool(name="const", bufs=1) as constp,
            tc.tile_pool(name="xtp", bufs=2) as xtp,
            tc.tile_pool(name="psp", bufs=2, space="PSUM") as psp,
            tc.tile_pool(name="evp", bufs=4) as evp,
            tc.tile_pool(name="finp", bufs=1) as finp,
        ):
            lhsT = constp.tile([CONTRACT, ND * KPD], FP16)
            nc.sync.dma_start(
                lhsT.rearrange("r (d m) -> r d m", d=ND),
                wt.ap().rearrange("d r m -> r d m"),
            )
            bias_sb = constp.tile([MT, ND * NKT], F32)
            nc.sync.dma_start(
                bias_sb[:, :], bia.ap().rearrange("d (kt p) -> p (d kt)", p=MT)
            )
            trash = constp.tile([MT, 2048], FP16)
            slots_mx = [
                constp.tile([MT, ND * NKT * 3], F32, name=f"smx{b}") for b in range(NB)
            ]
            slots_ct = [
                constp.tile([MT, ND * NKT * 3], F32, name=f"sct{b}") for b in range(NB)
            ]

            for di, d in enumerate(DILS):
                xt = xtp.tile([CONTRACT, NB * L], FP16, tag="xt", name=f"xt{di}")
                for b in range(NB):
                    base = b * L
                    for j in range(KLEN):
                        s = (j - 4) * d
                        t0 = max(0, -s)
                        t1 = min(L, L - s)
                        rows = slice(C * j, C * j + C)
                        nc.sync.dma_start(
                            xt[rows, base + t0 : base + t1], xh.ap()[b, :, t0 + s : t1 + s]
                        )
                        if t0 > 0:
                            nc.sync.dma_start(xt[rows, base : base + t0], zer.ap()[:, 0:t0])
                        if t1 < L:
                            nc.sync.dma_start(
                                xt[rows, base + t1 : base + L], zer.ap()[:, 0 : L - t1]
                            )
                for kt in range(NKT):
                    lhs = lhsT[:, di * KPD + kt * MT : di * KPD + kt * MT + MT]
                    bcol = di * NKT + kt
                    for b in range(NB):
                        for ch, (c0, c1) in enumerate(CHUNKS):
                            w = c1 - c0
                            pt = psp.tile(
                                [MT, 2048], F32, tag="ps", name=f"pt{di}_{kt}_{b}_{ch}"
                            )
                            for t in range(c0, c1, MM_N):
                                n = min(MM_N, c1 - t)
                                nc.tensor.matmul(
                                    pt[:, t - c0 : t - c0 + n],
                                    lhs,
                                    xt[:, b * L + t : b * L + t + n],
                                    start=True,
                                    stop=True,
                                )
                            ev = evp.tile(
                                [MT, 2048], FP16, tag="ev", name=f"ev{di}_{kt}_{b}_{ch}"
                            )
                            scol = bcol * 3 + ch
                            mxout = slots_mx[b][:, scol : scol + 1]
                            if ch < 2:
                                # ScalarE eviction, then 4x max on the fp16 copy
                                nc.scalar.copy(ev[:, :w], pt[:, :w])
                                nc.vector.tensor_scalar(
                                    trash[:, :w],
                                    ev[:, :w],
                                    0.0,
                                    None,
                                    op0=ALU.add,
                                    op1=ALU.max,
                                    accum_out=mxout,
                                )
                            else:
                                # VectorE fused evict+max straight from PSUM
                                nc.vector.tensor_scalar(
                                    ev[:, :w],
                                    pt[:, :w],
                                    0.0,
                                    None,
                                    op0=ALU.add,
                                    op1=ALU.max,
                                    accum_out=mxout,
                                )
                            nc.vector.tensor_scalar(
                                trash[:, :w],
                                ev[:, :w],
                                bias_sb[:, bcol : bcol + 1],
                                None,
                                op0=ALU.is_gt,
                                op1=ALU.add,
                                accum_out=slots_ct[b][:, scol : scol + 1],
                            )

            outv = out.ap().rearrange(
                "bb (d s kt p) -> bb p s d kt", d=ND, s=2, kt=NKT
            )
            for b in range(NB):
                mxr = finp.tile([MT, ND * NKT], F32, name=f"mxr{b}")
                nc.vector.tensor_reduce(
                    mxr[:, :],
                    slots_mx[b].rearrange("p (g c) -> p g c", c=3),
                    axis=mybir.AxisListType.X,
                    op=ALU.max,
                )
                ctr = finp.tile([MT, ND * NKT], F32, name=f"ctr{b}")
                nc.vector.tensor_reduce(
                    ctr[:, :],
                    slots_ct[b].rearrange("p (g c) -> p g c", c=3),
                    axis=mybir.AxisListType.X,
                    op=ALU.add,
                )
                ppv = finp.tile([MT, ND * NKT], F32, name=f"ppv{b}")
                nc.vector.tensor_scalar_mul(ppv[:, :], ctr[:, :], 1.0 / L)
                for di in range(ND):
                    nc.sync.dma_start(
                        outv[b, :, 0, di, :], mxr[:, di * NKT : (di + 1) * NKT]
                    )
                    nc.sync.dma_start(
                        outv[b, :, 1, di, :], ppv[:, di * NKT : (di + 1) * NKT]
                    )


_COMPILED = None


def get_compiled():
    global _COMPILED
    if _COMPILED is None:
        nc = bacc.Bacc(
            "TRN2", target_bir_lowering=False, debug=False, num_devices=NCORES
        )
        _emit(nc)
        nc.compile()
        _COMPILED = nc
    return _COMPILED


def make_in_maps(x, weights, biases):
    # W[d,k,c,j] -> wt[d, j*12+c, k], matching the Xs row order (j outer, c inner)
    wtr = np.ascontiguousarray(
        weights.astype(np.float16).transpose(0, 3, 2, 1).reshape(ND, CONTRACT, KPD)
    )
    bia = np.ascontiguousarray(biases.astype(np.float32))
    zer = np.zeros((C, 2048), np.float16)
    xh = x.astype(np.float16)
    maps = []
    for c in range(NCORES):
        maps.append(
            {
                "xh": np.ascontiguousarray(xh[NB * c : NB * (c + 1)]),
                "wt": wtr,
                "bia": bia,
                "zer": zer,
            }
        )
    return maps


def run(x, weights, biases, trace=False, **kw):
    nc = get_compiled()
    res = run_bass_kernel_spmd(
        nc, make_in_maps(x, weights, biases), core_ids=list(range(NCORES)),
        trace=trace, **kw
    )
    outs = np.concatenate([r["out"] for r in res.results], axis=0)
    return outs.astype(np.float32), res


def kernel(x, weights, biases):
    out, _ = run(x, weights, biases)
    return out


def bench(x, weights, biases, iters=20):
    """Time the sharded PJRT executable with pre-staged device inputs.

    Returns (out, per_call_wall_ns_list). Mirrors bass2jax.run_bass_via_pjrt's
    multi-core path, but stages inputs once and times repeated dispatches.
    """
    import time

    import jax
    import jax.numpy as jnp
    from jax.sharding import Mesh, NamedSharding, PartitionSpec
    from jax.experimental.shard_map import shard_map

    import concourse.bass2jax as b2j
    import concourse.mybir as mb

    nc = get_compiled()
    b2j.install_neuronx_cc_hook()
    in_maps = make_in_maps(x, weights, biases)

    partition_name = nc.partition_id_tensor.name if nc.partition_id_tensor else None
    in_names, out_names, out_avals, zero_outs = [], [], [], []
    for alloc in nc.m.functions[0].allocations:
        if not isinstance(alloc, mb.MemoryLocationSet):
            continue
        name = alloc.memorylocations[0].name
        if alloc.kind == "ExternalInput":
            if name != partition_name:
                in_names.append(name)
        elif alloc.kind == "ExternalOutput":
            out_names.append(name)
            shape = tuple(alloc.tensor_shape)
            dtype = mb.dt.np(alloc.dtype)
            out_avals.append(jax.core.ShapedArray(shape, dtype))
            zero_outs.append(np.zeros(shape, dtype))
    n_params = len(in_names)
    n_outs = len(out_avals)
    all_names = in_names + out_names
    if partition_name is not None:
        all_names = all_names + [partition_name]

    def _body(*args):
        operands = list(args)
        if partition_name is not None:
            operands.append(b2j.partition_id_tensor())
        outs = b2j._bass_exec_p.bind(
            *operands,
            out_avals=tuple(out_avals),
            in_names=tuple(all_names),
            out_names=tuple(out_names),
            lowering_input_output_aliases=(),
            sim_require_finite=True,
            sim_require_nnan=True,
            nc=nc,
        )
        return tuple(outs)

    devices = jax.devices()[:NCORES]
    mesh = Mesh(np.asarray(devices), ("core",))
    spec = PartitionSpec("core")
    sharded = jax.jit(
        shard_map(
            _body,
            mesh=mesh,
            in_specs=(spec,) * (n_params + n_outs),
            out_specs=(spec,) * n_outs,
            check_rep=False,
        ),
        donate_argnums=tuple(range(n_params, n_params + n_outs)),
        keep_unused=True,
    )
    sh = NamedSharding(mesh, spec)
    concat_in = [
        jax.device_put(
            np.concatenate([np.asarray(m[name]) for m in in_maps], axis=0), sh
        )
        for name in in_names
    ]
    zero_host = [np.zeros((NCORES * z.shape[0], *z.shape[1:]), z.dtype) for z in zero_outs]

    times = []
    out_arrs = None
    for i in range(iters + 1):
        zeros_dev = [jax.device_put(z, sh) for z in zero_host]
        jax.block_until_ready(zeros_dev)
        t0 = time.perf_counter()
        out_arrs = sharded(*concat_in, *zeros_dev)
        jax.block_until_ready(out_arrs)
        t1 = time.perf_counter()
        if i > 0:  # skip warmup/compile call
            times.append((t1 - t0) * 1e9)
    out = np.asarray(out_arrs[out_names.index("out")]).reshape(NCORES * NB, -1)
    return out.astype(np.float32), times
